# revision 7
# baseline (speedup 1.0000x reference)
"""Trainium2 Bass kernel for nn_AttentionBlock (GroupNorm + 1x1-conv QKV +
full self-attention over N=HW=4096 + output projection + residual).

Distribution: data-parallel over batch B=8, one batch element per NeuronCore.

Per-core layout / algorithm (C=128 channels on SBUF partitions, N=4096 free):
  1. GroupNorm stats via two ACT passes (Square + Identity, both with
     accum_out row-sums), cross-partition group combine via tiny indicator
     matmuls on the PE.
  2. hn = a_c * x + b_c  (ACT + DVE, output bf16).
  3. Q, K in natural [c, n] layout (lhsT = host-pretransposed weights, bf16
     so the FWL fast-weight-load path engages); V^T in [n, c] tile-major
     layout (lhsT = hn tiles).  A few dummy f32 matmuls run while the
     groupnorm scalar chain computes, so the PE HAM clock is warm before
     the QKV stream starts.
  4. Main loop (2 halves x 32 j-tiles), software-pipelined so the PE never
     waits on its own iteration's exp: emit S(j) -> exp(j) -> O(j-1):
       S^T tile = K_j^T Q  (PE, bf16 in / f32 PSUM out),
       P^T = exp(S^T) (ACT, -> bf16),
       acc += P^T (DVE bf16 2x-mode denominator partials),
       O += V^T_j^T P^T (PE accumulate in PSUM, bf16 operands).
     No max-subtraction: logits are ~N(0,1) so exp is safe.
  5. Tail per half: denominators via GPSIMD partition_all_reduce (cross-
     partition sum of acc, broadcast to all partitions -- no PSUM, no
     matmul), reciprocal_approx_fast on DVE (~18-bit), O_norm = O * recip
     (DVE), proj = w_proj^T O_norm (PE, borrowing the S PSUM tags),
     out = (x + b_eff) + proj, streamed to DRAM per 512-block.  The h0
     tail is interleaved into h1's main loop so only h1's tail is exposed.

Bias algebra: b_k is dropped entirely -- k_j = Wk hn_j + b_k adds q_i.b_k to
every logit of query i, a per-i constant that cancels exactly in softmax.
b_q folded into the Q PSUM->SBUF copy (DVE); b_v folded into
b_eff = b_proj + w_proj @ b_v (host precompute, exact).  The attention scale
C^-0.5 is folded into w_q/b_q on the host (exact reparameterization).

bf16 everywhere in attention: logit noise ~0.006 abs on N(0,1) logits and
0.4% weight noise post-softmax -- ~1e-3 relative on the final output vs the
2e-2 gate.
"""

import numpy as np

B, C, H, W = 8, 128, 64, 64
HW = H * W                      # 4096
GROUPS = 8
GSIZE = C // GROUPS             # 16
EPS = 1e-5
NJ = HW // 128                  # 32 j-tiles
IBLK = 512
NIB = HW // IBLK                # 8 i-blocks
NHALF = 2
HWID = HW // NHALF              # 2048
SCALE = float(C) ** -0.5

_CACHE = {}


def _build():
    from contextlib import ExitStack

    import concourse.bacc as bacc
    import concourse.tile as tile
    from concourse import bass_isa, mybir

    f32 = mybir.dt.float32
    bf16 = mybir.dt.bfloat16
    AF = mybir.ActivationFunctionType

    nc = bacc.Bacc("TRN2", target_bir_lowering=False, debug=False)

    x_in = nc.dram_tensor("x", [C, HW], f32, kind="ExternalInput")
    gamma_in = nc.dram_tensor("gamma", [C, 1], f32, kind="ExternalInput")
    beta_in = nc.dram_tensor("beta", [C, 1], f32, kind="ExternalInput")
    bq_in = nc.dram_tensor("bq", [C, 1], f32, kind="ExternalInput")
    beff_in = nc.dram_tensor("beff", [C, 1], f32, kind="ExternalInput")
    wq_in = nc.dram_tensor("wqT", [C, C], f32, kind="ExternalInput")
    wk_in = nc.dram_tensor("wkT", [C, C], f32, kind="ExternalInput")
    wv_in = nc.dram_tensor("wvT", [C, C], f32, kind="ExternalInput")
    wp_in = nc.dram_tensor("wpT", [C, C], f32, kind="ExternalInput")
    ig_in = nc.dram_tensor("ig", [C, GROUPS], f32, kind="ExternalInput")
    igt_in = nc.dram_tensor("igt", [GROUPS, C], f32, kind="ExternalInput")
    out_dram = nc.dram_tensor("out", [C, HW], f32, kind="ExternalOutput")

    with tile.TileContext(nc) as tc, ExitStack() as ctx, \
         nc.allow_low_precision(reason="bf16 attention pipeline; error "
                                "budget audited vs the 2e-2 gate"):
        const = ctx.enter_context(tc.tile_pool(name="const", bufs=1))
        big = ctx.enter_context(tc.tile_pool(name="big", bufs=1))
        stats = ctx.enter_context(tc.tile_pool(name="stats", bufs=1))
        ptpool = ctx.enter_context(tc.tile_pool(name="pt", bufs=3))
        stg = ctx.enter_context(tc.tile_pool(name="stage", bufs=2))

        # ---------------- load x on the sync+scalar queues; consts on the
        # vector queue so they don't serialize the x stream ----------------
        NCH = 4
        CHW = HW // NCH
        x_sb = big.tile([C, HW], f32, tag="x")
        for ch in range(NCH):
            sl = slice(ch * CHW, (ch + 1) * CHW)
            eng = nc.sync if ch % 2 == 0 else nc.scalar
            eng.dma_start(x_sb[:, sl], x_in[:, sl])

        def cload(t_in, shape, tag):
            t = const.tile(shape, f32, tag=tag)
            nc.gpsimd.dma_start(t[:], t_in[:])
            return t

        gamma = cload(gamma_in, [C, 1], "c_gamma")
        beta = cload(beta_in, [C, 1], "c_beta")
        bq = cload(bq_in, [C, 1], "c_bq")
        beff = cload(beff_in, [C, 1], "c_beff")
        ig = cload(ig_in, [C, GROUPS], "c_ig")
        igt = cload(igt_in, [GROUPS, C], "c_igt")
        wq_f = cload(wq_in, [C, C], "c_wq_f")
        wk_f = cload(wk_in, [C, C], "c_wk_f")
        wv_f = cload(wv_in, [C, C], "c_wv_f")
        wp_f = cload(wp_in, [C, C], "c_wp_f")

        wq = const.tile([C, C], bf16)
        nc.vector.tensor_copy(wq[:], wq_f[:])
        wk = const.tile([C, C], bf16)
        nc.vector.tensor_copy(wk[:], wk_f[:])
        wv = const.tile([C, C], bf16)
        nc.vector.tensor_copy(wv[:], wv_f[:])
        wp = const.tile([C, C], bf16)
        nc.vector.tensor_copy(wp[:], wp_f[:])

        eps_t = const.tile([GROUPS, 1], f32)
        nc.vector.memset(eps_t[:], EPS)
        magic_t = const.tile([GROUPS, 1], mybir.dt.uint32)
        nc.vector.memset(magic_t[:], 0x5F3759DF)
        c15_t = const.tile([GROUPS, 1], f32)
        nc.vector.memset(c15_t[:], 1.5)

        # ---------------- groupnorm stats (split across DVE and ACT) ----
        st2 = stats.tile([C, 2], f32)
        s2p = stats.tile([C, NCH], f32)
        s1p = stats.tile([C, NCH], f32)
        adum = stats.tile([C, CHW], f32)
        for ch in range(NCH):  # x^2 sums on ACT, x sums on DVE
            sl = slice(ch * CHW, (ch + 1) * CHW)
            nc.scalar.activation(
                adum[:], x_sb[:, sl], AF.Square, accum_out=s2p[:, ch:ch + 1]
            )
            nc.vector.reduce_sum(
                s1p[:, ch:ch + 1], x_sb[:, sl], axis=mybir.AxisListType.X
            )
        warm = stats.tile([GROUPS, 1], f32)
        nc.scalar.activation(warm[:], eps_t[:], AF.Exp)
        nc.vector.reduce_sum(st2[:, 1:2], s2p[:], axis=mybir.AxisListType.X)
        nc.vector.reduce_sum(st2[:, 0:1], s1p[:], axis=mybir.AxisListType.X)

        # PSUM layout for the whole kernel body: two S tiles (2 banks each,
        # independently released) + one O accumulator (4 banks). The QKV
        # rounds, groupnorm matmuls AND the projection tail all borrow the
        # S slots so there is no pool barrier anywhere.
        acc = big.tile([C, HW], bf16, tag="acc")
        o_sb = big.tile([C, HW], bf16, tag="o")
        out_sb = big.tile([C, HW], f32, tag="scratch")
        den = stats.tile([C, HW], f32)   # broadcast denominators
        rbc = big.tile([C, HW], f32, tag="rbc")
        HQ = HWID // 2  # 1024
        with tc.tile_pool(name="ps_s", bufs=1, space="PSUM") as ps_s, \
             tc.tile_pool(name="ps_o", bufs=1, space="PSUM") as ps_o:
            gs_ps = ps_s.tile([GROUPS, 2], f32, tag="s0")
            nc.tensor.matmul(gs_ps[:], ig[:], st2[:], start=True, stop=True)
            gstats = stats.tile([GROUPS, 2], f32)
            nc.vector.tensor_copy(gstats[:], gs_ps[:])
            inv_n = 1.0 / float(GSIZE * HW)
            gmean = stats.tile([GROUPS, 1], f32)
            nc.vector.tensor_scalar_mul(gmean[:], gstats[:, 0:1], inv_n)
            gm2 = stats.tile([GROUPS, 1], f32)
            nc.vector.tensor_scalar_mul(gm2[:], gstats[:, 1:2], inv_n)
            gmsq = stats.tile([GROUPS, 1], f32)
            nc.vector.tensor_mul(gmsq[:], gmean[:], gmean[:])
            gvar = stats.tile([GROUPS, 1], f32)
            nc.vector.tensor_sub(gvar[:], gm2[:], gmsq[:])
            gve = stats.tile([GROUPS, 1], f32)
            nc.vector.tensor_scalar(
                gve[:], gvar[:], eps_t[:], None, mybir.AluOpType.add
            )
            # warm the PE HAM clock while the scalar chain below runs: a few
            # dummy f32 matmuls keep the array streaming so the QKV rounds
            # start at full clock (results never read; slot reused later)
            wmt = ps_s.tile([C, HQ], f32, tag="s1")
            for _ in range(4):
                nc.tensor.matmul(
                    wmt[:, 0:IBLK], wq_f[:], x_sb[:, 0:IBLK],
                    start=True, stop=True,
                )
            # rstd = rsqrt(var+eps): quake initial guess + Newton steps (DVE
            # only -- ACT Sqrt/Ln would each force a ~1.3us table-set swap)
            u32 = mybir.dt.uint32
            gu = stats.tile([GROUPS, 1], u32)
            nc.vector.tensor_scalar(
                gu[:], gve[:].bitcast(u32), 1, None,
                mybir.AluOpType.logical_shift_right,
            )
            nc.vector.tensor_sub(gu[:], magic_t[:], gu[:])
            gy = stats.tile([GROUPS, 1], f32)
            nc.vector.tensor_copy(gy[:], gu[:].bitcast(f32))
            gh = stats.tile([GROUPS, 1], f32)
            nc.vector.tensor_scalar_mul(gh[:], gve[:], 0.5)
            gt = stats.tile([GROUPS, 1], f32)
            for _ in range(2):
                nc.vector.tensor_mul(gt[:], gy[:], gy[:])
                nc.vector.tensor_mul(gt[:], gt[:], gh[:])
                nc.vector.tensor_sub(gt[:], c15_t[:], gt[:])
                nc.vector.tensor_mul(gy[:], gy[:], gt[:])
            gmr = stats.tile([GROUPS, 2], f32)
            nc.vector.tensor_copy(gmr[:, 1:2], gy[:])
            nc.vector.tensor_copy(gmr[:, 0:1], gmean[:])

            bc_ps = ps_s.tile([C, 2], f32, tag="s0")
            nc.tensor.matmul(bc_ps[:], igt[:], gmr[:], start=True, stop=True)
            a_c = stats.tile([C, 1], f32)
            b_c = stats.tile([C, 1], f32)
            tmc = stats.tile([C, 1], f32)
            nc.vector.tensor_scalar_mul(a_c[:], gamma[:], bc_ps[:, 1:2])
            nc.vector.tensor_scalar_mul(tmc[:], a_c[:], bc_ps[:, 0:1])
            nc.vector.tensor_sub(b_c[:], beta[:], tmc[:])

            hn = big.tile([C, HW], bf16, tag="hn")
            q_r = big.tile([C, HW], bf16, tag="q")
            k_r = big.tile([C, HW], bf16, tag="k")
            vt = big.tile([C, NJ, C], bf16, tag="vt")

            def emit_hn(h, engine):
                hs = slice(h * HWID, (h + 1) * HWID)
                if engine == "act":
                    nc.scalar.activation(
                        hn[:, hs], x_sb[:, hs], AF.Identity, bias=b_c[:], scale=a_c[:]
                    )
                else:
                    nc.vector.tensor_scalar(
                        hn[:, hs], x_sb[:, hs], a_c[:], b_c[:],
                        mybir.AluOpType.mult, mybir.AluOpType.add,
                    )

            def emit_k_round(h, r):  # r in 0..1, [C, HQ] rounds
                kp = ps_s.tile([C, HQ], f32, tag=f"s{r % 2}")
                for kk in range(2):
                    off = h * HWID + r * HQ + kk * IBLK
                    nc.tensor.matmul(
                        kp[:, kk * IBLK:(kk + 1) * IBLK], wk[:],
                        hn[:, off:off + IBLK], start=True, stop=True,
                    )
                # b_k dropped: a per-query constant in the logits, cancels in
                # softmax exactly.
                nc.vector.tensor_copy(
                    k_r[:, h * HWID + r * HQ:h * HWID + (r + 1) * HQ], kp[:]
                )

            def emit_q_round(h, r):
                qp = ps_s.tile([C, HQ], f32, tag=f"s{r % 2}")
                for kk in range(2):
                    off = h * HWID + r * HQ + kk * IBLK
                    nc.tensor.matmul(
                        qp[:, kk * IBLK:(kk + 1) * IBLK], wq[:],
                        hn[:, off:off + IBLK], start=True, stop=True,
                    )
                nc.vector.tensor_scalar(
                    q_r[:, h * HWID + r * HQ:h * HWID + (r + 1) * HQ], qp[:],
                    bq[:], None, mybir.AluOpType.add,
                )

            def emit_v_round(h, r):  # r in 0..3, 4 n-tiles per round
                vp = ps_s.tile([C, 4, C], f32, tag=f"s{r % 2}")
                for t in range(4):
                    nt = h * 16 + r * 4 + t
                    nc.tensor.matmul(
                        vp[:, t, :], hn[:, nt * 128:(nt + 1) * 128], wv[:],
                        start=True, stop=True,
                    )
                tsl = slice(h * 16 + r * 4, h * 16 + (r + 1) * 4)
                nc.vector.tensor_copy(vt[:, tsl, :], vp[:])

            emit_hn(0, "act")
            for r in range(2):
                emit_k_round(0, r)
            for r in range(2):
                emit_q_round(0, r)
            emit_hn(1, "dve")
            for r in range(4):
                emit_v_round(0, r)
            for r in range(2):
                emit_k_round(1, r)
            for r in range(4):
                emit_v_round(1, r)

            # ------------- denominator + projection tail emitters --------
            def emit_denom(h):
                # cross-partition sum of acc for this half, broadcast to all
                # partitions (GPSIMD, SBUF only) then 1/x on DVE (~18 bits)
                hs = slice(h * HWID, (h + 1) * HWID)
                nc.gpsimd.partition_all_reduce(
                    den[:, hs], acc[:, hs], 128, bass_isa.ReduceOp.add
                )
                nc.vector.reciprocal_approx_fast(rbc[:, hs], den[:, hs])

            def emit_onorm(h):
                hs = slice(h * HWID, (h + 1) * HWID)
                o_nrm = stg.tile([C, HWID], bf16, tag="onrm")
                nc.vector.tensor_mul(o_nrm[:], o_sb[:, hs], rbc[:, hs])
                return o_nrm

            def emit_proj(ib, o_nrm, base):
                # ib is the global 512-block index; base = offset in o_nrm
                sl = slice(ib * IBLK, (ib + 1) * IBLK)
                bt = ps_s.tile([C, HQ], f32, tag=f"s{ib % 2}")
                pp = bt[:, 0:IBLK]
                nc.tensor.matmul(
                    pp, wp[:], o_nrm[:, base:base + IBLK], start=True, stop=True
                )
                nc.vector.tensor_scalar(
                    out_sb[:, sl], pp, beff[:], None, mybir.AluOpType.add
                )
                # residual add: GPSIMD for blocks hidden under the main
                # loop, DVE for the latency-critical final blocks
                if ib < 4:
                    nc.gpsimd.tensor_add(out_sb[:, sl], out_sb[:, sl], x_sb[:, sl])
                else:
                    nc.vector.tensor_add(out_sb[:, sl], out_sb[:, sl], x_sb[:, sl])
                nc.scalar.dma_start(out_dram[:, sl], out_sb[:, sl])

            # ---------------- main attention loop ----------------
            # Software-pipelined: iteration j emits S(j) matmuls, exp(j),
            # then O(j-1), so the PE never stalls on its own iteration's exp.
            h0_onrm = [None]

            def tail_step(h, j):
                # interleave h0's tail into h1's loop
                if h != 1:
                    return
                if j == 1:
                    emit_denom(0)
                elif j == 5:
                    h0_onrm[0] = emit_onorm(0)
                elif j in (9, 13, 17, 21):
                    ib = (j - 9) // 4
                    emit_proj(ib, h0_onrm[0], ib * IBLK)

            for h in range(NHALF):
                hsl = slice(h * HWID, (h + 1) * HWID)
                if h == 1:
                    for r in range(2):
                        emit_q_round(1, r)
                op = ps_o.tile([C, HWID], f32, tag="o_ps")
                prev = None
                for j in range(NJ):
                    pt = ptpool.tile([C, HWID], bf16)
                    for half in range(2):
                        sp = ps_s.tile([C, HQ], f32, tag=f"s{half}")
                        for kk in range(2):
                            qoff = h * HWID + half * HQ + kk * IBLK
                            nc.tensor.matmul(
                                sp[:, kk * IBLK:(kk + 1) * IBLK],
                                k_r[:, j * 128:(j + 1) * 128],
                                q_r[:, qoff:qoff + IBLK],
                                start=True, stop=True,
                            )
                        nc.scalar.activation(
                            pt[:, half * HQ:(half + 1) * HQ], sp[:], AF.Exp
                        )
                    if prev is not None:
                        for kk in range(4):
                            sl = slice(kk * IBLK, (kk + 1) * IBLK)
                            nc.tensor.matmul(
                                op[:, sl], vt[:, j - 1, :], prev[:, sl],
                                start=(j == 1), stop=False,
                            )
                    if j == 0:
                        nc.vector.tensor_copy(acc[:, hsl], pt[:])
                    else:
                        nc.vector.tensor_add(acc[:, hsl], acc[:, hsl], pt[:])
                    prev = pt
                    tail_step(h, j)
                for kk in range(4):
                    sl = slice(kk * IBLK, (kk + 1) * IBLK)
                    nc.tensor.matmul(
                        op[:, sl], vt[:, NJ - 1, :], prev[:, sl],
                        start=False, stop=True,
                    )
                nc.vector.tensor_copy(o_sb[:, hsl], op[:])

            # h1 tail (h0's was interleaved above)
            emit_denom(1)
            o_nrm1 = emit_onorm(1)
            for ib in range(4, 8):
                emit_proj(ib, o_nrm1, (ib - 4) * IBLK)

    nc.compile()
    return nc


def _get_nc():
    if "nc" not in _CACHE:
        _CACHE["nc"] = _build()
    return _CACHE["nc"]


def _prep_inputs(x, gamma, beta, w_qkv, b_qkv, w_proj, b_proj):
    x = np.ascontiguousarray(x, dtype=np.float32)
    w_qkv = np.asarray(w_qkv, dtype=np.float32)
    b_qkv = np.asarray(b_qkv, dtype=np.float32)
    w_proj = np.asarray(w_proj, dtype=np.float32)
    b_proj = np.asarray(b_proj, dtype=np.float32)

    wq = w_qkv[0:C, :]
    wk = w_qkv[C:2 * C, :]
    wv = w_qkv[2 * C:3 * C, :]
    bqv = b_qkv[0:C]
    bvv = b_qkv[2 * C:3 * C]

    wqT = np.ascontiguousarray((wq * SCALE).T)
    wkT = np.ascontiguousarray(wk.T)
    wvT = np.ascontiguousarray(wv.T)
    wpT = np.ascontiguousarray(w_proj.T)
    beff = (b_proj + w_proj @ bvv).astype(np.float32)

    ig = np.zeros((C, GROUPS), np.float32)
    ig[np.arange(C), np.arange(C) // GSIZE] = 1.0
    igt = np.ascontiguousarray(ig.T)

    common = {
        "gamma": np.asarray(gamma, np.float32).reshape(C, 1),
        "beta": np.asarray(beta, np.float32).reshape(C, 1),
        "bq": (bqv * SCALE).reshape(C, 1),
        "beff": beff.reshape(C, 1),
        "wqT": wqT,
        "wkT": wkT,
        "wvT": wvT,
        "wpT": wpT,
        "ig": ig,
        "igt": igt,
    }
    in_maps = []
    for b in range(B):
        m = dict(common)
        m["x"] = np.ascontiguousarray(x[b].reshape(C, HW))
        in_maps.append(m)
    return in_maps


def kernel(x, gamma, beta, w_qkv, b_qkv, w_proj, b_proj):
    from concourse.bass_utils import run_bass_kernel_spmd

    nc = _get_nc()
    in_maps = _prep_inputs(x, gamma, beta, w_qkv, b_qkv, w_proj, b_proj)
    res = run_bass_kernel_spmd(nc, in_maps, list(range(B)))
    out = np.stack([res.results[b]["out"] for b in range(B)], axis=0)
    return out.reshape(B, C, H, W).astype(np.float32)


# revision 9
# speedup vs baseline: 1.2993x; 1.2993x over previous
"""Trainium2 Bass kernel for nn_AttentionBlock (GroupNorm + 1x1-conv QKV +
full self-attention over N=HW=4096 + output projection + residual).

Distribution: data-parallel over batch B=8, one batch element per NeuronCore.

Per-core layout / algorithm (C=128 channels on SBUF partitions, N=4096 free):
  1. GroupNorm stats via two ACT passes (Square + Identity, both with
     accum_out row-sums), cross-partition group combine via tiny indicator
     matmuls on the PE.
  2. hn = a_c * x + b_c  (ACT + DVE, output bf16).
  3. Q, K in natural [c, n] layout (lhsT = host-pretransposed weights, bf16
     so the FWL fast-weight-load path engages); V^T in [n, c] tile-major
     layout (lhsT = hn tiles).  A few dummy f32 matmuls run while the
     groupnorm scalar chain computes, so the PE HAM clock is warm before
     the QKV stream starts.
  4. Main loop (2 halves x 32 j-tiles), software-pipelined so the PE never
     waits on its own iteration's exp: emit S(j) -> exp(j) -> O(j-1):
       S^T tile = K_j^T Q  (PE, bf16 in / f32 PSUM out),
       P^T = exp(S^T) (ACT, -> bf16),
       acc += P^T (DVE bf16 2x-mode denominator partials),
       O += V^T_j^T P^T (PE accumulate in PSUM, bf16 operands).
     No max-subtraction: logits are ~N(0,1) so exp is safe.
  5. Tail per half: denominators via a broadcast ones-matmul (every out
     partition gets the column sum of acc), reciprocal_approx_fast on DVE
     straight off PSUM (~18-bit), O_norm = O * recip
     (DVE), proj = w_proj^T O_norm (PE, borrowing the S PSUM tags),
     out = (x + b_eff) + proj, streamed to DRAM per 512-block.  The h0
     tail is interleaved into h1's main loop so only h1's tail is exposed.

Bias algebra: b_k is dropped entirely -- k_j = Wk hn_j + b_k adds q_i.b_k to
every logit of query i, a per-i constant that cancels exactly in softmax.
b_q folded into the Q PSUM->SBUF copy (DVE); b_v folded into
b_eff = b_proj + w_proj @ b_v (host precompute, exact).  The attention scale
C^-0.5 is folded into w_q/b_q on the host (exact reparameterization).

bf16 everywhere in attention: logit noise ~0.006 abs on N(0,1) logits and
0.4% weight noise post-softmax -- ~1e-3 relative on the final output vs the
2e-2 gate.
"""

import numpy as np

B, C, H, W = 8, 128, 64, 64
HW = H * W                      # 4096
GROUPS = 8
GSIZE = C // GROUPS             # 16
EPS = 1e-5
NJ = HW // 128                  # 32 j-tiles
IBLK = 512
NIB = HW // IBLK                # 8 i-blocks
NHALF = 2
HWID = HW // NHALF              # 2048
SCALE = float(C) ** -0.5

_CACHE = {}


def _build():
    from contextlib import ExitStack

    import concourse.bacc as bacc
    import concourse.tile as tile
    from concourse import mybir

    f32 = mybir.dt.float32
    bf16 = mybir.dt.bfloat16
    AF = mybir.ActivationFunctionType

    nc = bacc.Bacc("TRN2", target_bir_lowering=False, debug=False)

    x_in = nc.dram_tensor("x", [C, HW], f32, kind="ExternalInput")
    gamma_in = nc.dram_tensor("gamma", [C, 1], f32, kind="ExternalInput")
    beta_in = nc.dram_tensor("beta", [C, 1], f32, kind="ExternalInput")
    bq_in = nc.dram_tensor("bq", [C, 1], f32, kind="ExternalInput")
    beff_in = nc.dram_tensor("beff", [C, 1], f32, kind="ExternalInput")
    wq_in = nc.dram_tensor("wqT", [C, C], f32, kind="ExternalInput")
    wk_in = nc.dram_tensor("wkT", [C, C], f32, kind="ExternalInput")
    wv_in = nc.dram_tensor("wvT", [C, C], f32, kind="ExternalInput")
    wp_in = nc.dram_tensor("wpT", [C, C], f32, kind="ExternalInput")
    ig_in = nc.dram_tensor("ig", [C, GROUPS], f32, kind="ExternalInput")
    igt_in = nc.dram_tensor("igt", [GROUPS, C], f32, kind="ExternalInput")
    out_dram = nc.dram_tensor("out", [C, HW], f32, kind="ExternalOutput")

    with tile.TileContext(nc) as tc, ExitStack() as ctx, \
         nc.allow_low_precision(reason="bf16 attention pipeline; error "
                                "budget audited vs the 2e-2 gate"):
        const = ctx.enter_context(tc.tile_pool(name="const", bufs=1))
        big = ctx.enter_context(tc.tile_pool(name="big", bufs=1))
        stats = ctx.enter_context(tc.tile_pool(name="stats", bufs=1))
        ptpool = ctx.enter_context(tc.tile_pool(name="pt", bufs=3))
        stg = ctx.enter_context(tc.tile_pool(name="stage", bufs=2))

        # ---------------- load x on the sync+scalar queues; consts on the
        # vector queue so they don't serialize the x stream ----------------
        NCH = 4
        CHW = HW // NCH
        x_sb = big.tile([C, HW], f32, tag="x")
        for ch in range(NCH):
            sl = slice(ch * CHW, (ch + 1) * CHW)
            eng = nc.sync if ch % 2 == 0 else nc.scalar
            eng.dma_start(x_sb[:, sl], x_in[:, sl])

        def cload(t_in, shape, tag):
            t = const.tile(shape, f32, tag=tag)
            nc.sync.dma_start(t[:], t_in[:])
            return t

        gamma = cload(gamma_in, [C, 1], "c_gamma")
        beta = cload(beta_in, [C, 1], "c_beta")
        bq = cload(bq_in, [C, 1], "c_bq")
        beff = cload(beff_in, [C, 1], "c_beff")
        ig = cload(ig_in, [C, GROUPS], "c_ig")
        igt = cload(igt_in, [GROUPS, C], "c_igt")
        wq_f = cload(wq_in, [C, C], "c_wq_f")
        wk_f = cload(wk_in, [C, C], "c_wk_f")
        wv_f = cload(wv_in, [C, C], "c_wv_f")
        wp_f = cload(wp_in, [C, C], "c_wp_f")

        wq = const.tile([C, C], bf16)
        nc.vector.tensor_copy(wq[:], wq_f[:])
        wk = const.tile([C, C], bf16)
        nc.vector.tensor_copy(wk[:], wk_f[:])
        wv = const.tile([C, C], bf16)
        nc.vector.tensor_copy(wv[:], wv_f[:])
        wp = const.tile([C, C], bf16)
        nc.vector.tensor_copy(wp[:], wp_f[:])

        ones_bc = const.tile([C, C], bf16)
        nc.vector.memset(ones_bc[:], 1.0)

        eps_t = const.tile([GROUPS, 1], f32)
        nc.vector.memset(eps_t[:], EPS)
        magic_t = const.tile([GROUPS, 1], mybir.dt.uint32)
        nc.vector.memset(magic_t[:], 0x5F3759DF)
        c15_t = const.tile([GROUPS, 1], f32)
        nc.vector.memset(c15_t[:], 1.5)

        # ---------------- groupnorm stats (split across DVE and ACT) ----
        st2 = stats.tile([C, 2], f32)
        s2p = stats.tile([C, NCH], f32)
        s1p = stats.tile([C, NCH], f32)
        adum = stats.tile([C, CHW], f32)
        for ch in range(NCH):  # x^2 sums on ACT, x sums on DVE
            sl = slice(ch * CHW, (ch + 1) * CHW)
            nc.scalar.activation(
                adum[:], x_sb[:, sl], AF.Square, accum_out=s2p[:, ch:ch + 1]
            )
            nc.vector.reduce_sum(
                s1p[:, ch:ch + 1], x_sb[:, sl], axis=mybir.AxisListType.X
            )
        warm = stats.tile([GROUPS, 1], f32)
        nc.scalar.activation(warm[:], eps_t[:], AF.Exp)
        nc.vector.reduce_sum(st2[:, 1:2], s2p[:], axis=mybir.AxisListType.X)
        nc.vector.reduce_sum(st2[:, 0:1], s1p[:], axis=mybir.AxisListType.X)

        # PSUM layout for the whole kernel body: two S tiles (2 banks each,
        # independently released) + one O accumulator (4 banks). The QKV
        # rounds, groupnorm matmuls AND the projection tail all borrow the
        # S slots so there is no pool barrier anywhere.
        acc0 = big.tile([C, HWID], bf16, tag="acc0")
        acc1 = big.tile([C, HWID], bf16, tag="acc1")
        acch = [acc0, acc1]
        o_sb0 = big.tile([C, HWID], bf16, tag="o0")
        o_sb1 = big.tile([C, HWID], bf16, tag="o1")
        o_sbh = [o_sb0, o_sb1]
        out_sb = big.tile([C, HW], f32, tag="scratch")
        rbc0 = big.tile([C, HWID], f32, tag="rbc0")
        rbc1 = big.tile([C, HWID], f32, tag="rbc1")
        rbch = [rbc0, rbc1]
        HQ = HWID // 2  # 1024
        with tc.tile_pool(name="ps_s", bufs=1, space="PSUM") as ps_s, \
             tc.tile_pool(name="ps_o", bufs=1, space="PSUM") as ps_o:
            gs_ps = ps_s.tile([GROUPS, 2], f32, tag="s0")
            nc.tensor.matmul(gs_ps[:], ig[:], st2[:], start=True, stop=True)
            gstats = stats.tile([GROUPS, 2], f32)
            nc.vector.tensor_copy(gstats[:], gs_ps[:])
            inv_n = 1.0 / float(GSIZE * HW)
            gmean = stats.tile([GROUPS, 1], f32)
            nc.vector.tensor_scalar_mul(gmean[:], gstats[:, 0:1], inv_n)
            gm2 = stats.tile([GROUPS, 1], f32)
            nc.vector.tensor_scalar_mul(gm2[:], gstats[:, 1:2], inv_n)
            gmsq = stats.tile([GROUPS, 1], f32)
            nc.vector.tensor_mul(gmsq[:], gmean[:], gmean[:])
            gvar = stats.tile([GROUPS, 1], f32)
            nc.vector.tensor_sub(gvar[:], gm2[:], gmsq[:])
            gve = stats.tile([GROUPS, 1], f32)
            nc.vector.tensor_scalar(
                gve[:], gvar[:], eps_t[:], None, mybir.AluOpType.add
            )
            # warm the PE HAM clock while the scalar chain below runs: a few
            # dummy f32 matmuls keep the array streaming so the QKV rounds
            # start at full clock (results never read; slot reused later)
            wmt = ps_s.tile([C, HQ], f32, tag="s1")
            for _ in range(6):
                nc.tensor.matmul(
                    wmt[:, 0:IBLK], wq_f[:], x_sb[:, 0:IBLK],
                    start=True, stop=True,
                )
            # rstd = rsqrt(var+eps): quake initial guess + Newton steps (DVE
            # only -- ACT Sqrt/Ln would each force a ~1.3us table-set swap)
            u32 = mybir.dt.uint32
            gu = stats.tile([GROUPS, 1], u32)
            nc.vector.tensor_scalar(
                gu[:], gve[:].bitcast(u32), 1, None,
                mybir.AluOpType.logical_shift_right,
            )
            nc.vector.tensor_sub(gu[:], magic_t[:], gu[:])
            gy = stats.tile([GROUPS, 1], f32)
            nc.vector.tensor_copy(gy[:], gu[:].bitcast(f32))
            gh = stats.tile([GROUPS, 1], f32)
            nc.vector.tensor_scalar_mul(gh[:], gve[:], 0.5)
            gt = stats.tile([GROUPS, 1], f32)
            for _ in range(2):
                nc.vector.tensor_mul(gt[:], gy[:], gy[:])
                nc.vector.tensor_mul(gt[:], gt[:], gh[:])
                nc.vector.tensor_sub(gt[:], c15_t[:], gt[:])
                nc.vector.tensor_mul(gy[:], gy[:], gt[:])
            gmr = stats.tile([GROUPS, 2], f32)
            nc.vector.tensor_copy(gmr[:, 1:2], gy[:])
            nc.vector.tensor_copy(gmr[:, 0:1], gmean[:])

            bc_ps = ps_s.tile([C, 2], f32, tag="s0")
            nc.tensor.matmul(bc_ps[:], igt[:], gmr[:], start=True, stop=True)
            a_c = stats.tile([C, 1], f32)
            b_c = stats.tile([C, 1], f32)
            tmc = stats.tile([C, 1], f32)
            nc.vector.tensor_scalar_mul(a_c[:], gamma[:], bc_ps[:, 1:2])
            nc.vector.tensor_scalar_mul(tmc[:], a_c[:], bc_ps[:, 0:1])
            nc.vector.tensor_sub(b_c[:], beta[:], tmc[:])

            hn = big.tile([C, HW], bf16, tag="hn")
            q_r = big.tile([C, HW], bf16, tag="q")
            k_r = big.tile([C, HW], bf16, tag="k")
            vt = big.tile([C, NJ, C], bf16, tag="vt")

            def emit_hn(h, engine):
                hs = slice(h * HWID, (h + 1) * HWID)
                if engine == "act":
                    nc.scalar.activation(
                        hn[:, hs], x_sb[:, hs], AF.Identity, bias=b_c[:], scale=a_c[:]
                    )
                else:
                    nc.vector.tensor_scalar(
                        hn[:, hs], x_sb[:, hs], a_c[:], b_c[:],
                        mybir.AluOpType.mult, mybir.AluOpType.add,
                    )

            def emit_k_round(h, r):  # r in 0..1, [C, HQ] rounds
                kp = ps_s.tile([C, HQ], f32, tag=f"s{r % 2}")
                for kk in range(2):
                    off = h * HWID + r * HQ + kk * IBLK
                    nc.tensor.matmul(
                        kp[:, kk * IBLK:(kk + 1) * IBLK], wk[:],
                        hn[:, off:off + IBLK], start=True, stop=True,
                    )
                # b_k dropped: a per-query constant in the logits, cancels in
                # softmax exactly.
                nc.vector.tensor_copy(
                    k_r[:, h * HWID + r * HQ:h * HWID + (r + 1) * HQ], kp[:]
                )

            def emit_q_round(h, r):
                qp = ps_s.tile([C, HQ], f32, tag=f"s{r % 2}")
                for kk in range(2):
                    off = h * HWID + r * HQ + kk * IBLK
                    nc.tensor.matmul(
                        qp[:, kk * IBLK:(kk + 1) * IBLK], wq[:],
                        hn[:, off:off + IBLK], start=True, stop=True,
                    )
                nc.vector.tensor_scalar(
                    q_r[:, h * HWID + r * HQ:h * HWID + (r + 1) * HQ], qp[:],
                    bq[:], None, mybir.AluOpType.add,
                )

            def emit_v_round(h, r):  # r in 0..3, 4 n-tiles per round
                vp = ps_s.tile([C, 4, C], f32, tag=f"s{r % 2}")
                for t in range(4):
                    nt = h * 16 + r * 4 + t
                    nc.tensor.matmul(
                        vp[:, t, :], hn[:, nt * 128:(nt + 1) * 128], wv[:],
                        start=True, stop=True,
                    )
                tsl = slice(h * 16 + r * 4, h * 16 + (r + 1) * 4)
                nc.vector.tensor_copy(vt[:, tsl, :], vp[:])

            emit_hn(0, "act")
            for r in range(2):
                emit_k_round(0, r)
            for r in range(2):
                emit_q_round(0, r)
            emit_hn(1, "dve")
            for r in range(4):
                emit_v_round(0, r)
            for r in range(2):
                emit_k_round(1, r)
            for r in range(4):
                emit_v_round(1, r)

            # ------------- denominator + projection tail emitters --------
            def emit_denom(h, q):
                # cross-partition sum of one [C, HQ] quarter of acc via a
                # broadcast ones-matmul (every out partition = column sum),
                # then 1/x straight off PSUM on DVE (~18 bits correct)
                qs = slice(q * HQ, (q + 1) * HQ)
                dt_ = ps_s.tile([C, HQ], f32, tag=f"s{q % 2}")
                for kk in range(2):
                    ds = slice(q * HQ + kk * IBLK, q * HQ + (kk + 1) * IBLK)
                    nc.tensor.matmul(
                        dt_[:, kk * IBLK:(kk + 1) * IBLK], ones_bc[:],
                        acch[h][:, ds], start=True, stop=True,
                    )
                nc.vector.reciprocal_approx_fast(rbch[h][:, qs], dt_[:])

            def emit_onorm(h):
                o_nrm = stg.tile([C, HWID], bf16, tag="onrm")
                nc.vector.tensor_mul(o_nrm[:], o_sbh[h][:], rbch[h][:])
                return o_nrm

            def emit_proj(ib, o_nrm, base):
                # ib is the global 512-block index; base = offset in o_nrm
                sl = slice(ib * IBLK, (ib + 1) * IBLK)
                bt = ps_s.tile([C, HQ], f32, tag=f"s{ib % 2}")
                pp = bt[:, 0:IBLK]
                nc.tensor.matmul(
                    pp, wp[:], o_nrm[:, base:base + IBLK], start=True, stop=True
                )
                nc.vector.tensor_scalar(
                    out_sb[:, sl], pp, beff[:], None, mybir.AluOpType.add
                )
                # residual add: GPSIMD for blocks hidden under the main
                # loop, DVE for the latency-critical final blocks
                if ib < 4:
                    nc.gpsimd.tensor_add(out_sb[:, sl], out_sb[:, sl], x_sb[:, sl])
                else:
                    nc.vector.tensor_add(out_sb[:, sl], out_sb[:, sl], x_sb[:, sl])
                nc.scalar.dma_start(out_dram[:, sl], out_sb[:, sl])

            # ---------------- main attention loop ----------------
            # Software-pipelined: iteration j emits S(j) matmuls, exp(j),
            # then O(j-1), so the PE never stalls on its own iteration's exp.
            h0_onrm = [None]

            def tail_step(h, j):
                # interleave h0's tail into h1's loop (emitted before the
                # acc add of iteration j so the DVE reaches the PSUM reads
                # quickly and releases the borrowed S tag)
                if h != 1:
                    return
                if j == 3:
                    emit_denom(0, 0)
                elif j == 5:
                    emit_denom(0, 1)
                elif j == 7:
                    h0_onrm[0] = emit_onorm(0)
                elif j in (11, 15, 19, 23):
                    ib = (j - 11) // 4
                    emit_proj(ib, h0_onrm[0], ib * IBLK)

            for h in range(NHALF):
                if h == 1:
                    for r in range(2):
                        emit_q_round(1, r)
                op = ps_o.tile([C, HWID], f32, tag="o_ps")
                prev = None
                for j in range(NJ):
                    pt = ptpool.tile([C, HWID], bf16)
                    for half in range(2):
                        sp = ps_s.tile([C, HQ], f32, tag=f"s{half}")
                        for kk in range(2):
                            qoff = h * HWID + half * HQ + kk * IBLK
                            nc.tensor.matmul(
                                sp[:, kk * IBLK:(kk + 1) * IBLK],
                                k_r[:, j * 128:(j + 1) * 128],
                                q_r[:, qoff:qoff + IBLK],
                                start=True, stop=True,
                            )
                        nc.scalar.activation(
                            pt[:, half * HQ:(half + 1) * HQ], sp[:], AF.Exp
                        )
                    if prev is not None:
                        for kk in range(4):
                            sl = slice(kk * IBLK, (kk + 1) * IBLK)
                            nc.tensor.matmul(
                                op[:, sl], vt[:, j - 1, :], prev[:, sl],
                                start=(j == 1), stop=False,
                            )
                    tail_step(h, j)
                    if j == 0:
                        nc.vector.tensor_copy(acch[h][:], pt[:])
                    else:
                        nc.vector.tensor_add(acch[h][:], acch[h][:], pt[:])
                    prev = pt
                for kk in range(4):
                    sl = slice(kk * IBLK, (kk + 1) * IBLK)
                    nc.tensor.matmul(
                        op[:, sl], vt[:, NJ - 1, :], prev[:, sl],
                        start=False, stop=True,
                    )
                nc.vector.tensor_copy(o_sbh[h][:], op[:])

            # h1 tail (h0's was interleaved above)
            emit_denom(1, 0)
            emit_denom(1, 1)
            o_nrm1 = emit_onorm(1)
            for ib in range(4, 8):
                emit_proj(ib, o_nrm1, (ib - 4) * IBLK)

    nc.compile()
    return nc


def _get_nc():
    if "nc" not in _CACHE:
        _CACHE["nc"] = _build()
    return _CACHE["nc"]


def _prep_inputs(x, gamma, beta, w_qkv, b_qkv, w_proj, b_proj):
    x = np.ascontiguousarray(x, dtype=np.float32)
    w_qkv = np.asarray(w_qkv, dtype=np.float32)
    b_qkv = np.asarray(b_qkv, dtype=np.float32)
    w_proj = np.asarray(w_proj, dtype=np.float32)
    b_proj = np.asarray(b_proj, dtype=np.float32)

    wq = w_qkv[0:C, :]
    wk = w_qkv[C:2 * C, :]
    wv = w_qkv[2 * C:3 * C, :]
    bqv = b_qkv[0:C]
    bvv = b_qkv[2 * C:3 * C]

    wqT = np.ascontiguousarray((wq * SCALE).T)
    wkT = np.ascontiguousarray(wk.T)
    wvT = np.ascontiguousarray(wv.T)
    wpT = np.ascontiguousarray(w_proj.T)
    beff = (b_proj + w_proj @ bvv).astype(np.float32)

    ig = np.zeros((C, GROUPS), np.float32)
    ig[np.arange(C), np.arange(C) // GSIZE] = 1.0
    igt = np.ascontiguousarray(ig.T)

    common = {
        "gamma": np.asarray(gamma, np.float32).reshape(C, 1),
        "beta": np.asarray(beta, np.float32).reshape(C, 1),
        "bq": (bqv * SCALE).reshape(C, 1),
        "beff": beff.reshape(C, 1),
        "wqT": wqT,
        "wkT": wkT,
        "wvT": wvT,
        "wpT": wpT,
        "ig": ig,
        "igt": igt,
    }
    in_maps = []
    for b in range(B):
        m = dict(common)
        m["x"] = np.ascontiguousarray(x[b].reshape(C, HW))
        in_maps.append(m)
    return in_maps


def kernel(x, gamma, beta, w_qkv, b_qkv, w_proj, b_proj):
    from concourse.bass_utils import run_bass_kernel_spmd

    nc = _get_nc()
    in_maps = _prep_inputs(x, gamma, beta, w_qkv, b_qkv, w_proj, b_proj)
    res = run_bass_kernel_spmd(nc, in_maps, list(range(B)))
    out = np.stack([res.results[b]["out"] for b in range(B)], axis=0)
    return out.reshape(B, C, H, W).astype(np.float32)


# revision 19
# speedup vs baseline: 1.3588x; 1.0458x over previous
"""Trainium2 Bass kernel for nn_AttentionBlock (GroupNorm + 1x1-conv QKV +
full self-attention over N=HW=4096 + output projection + residual).

Distribution: data-parallel over batch B=8, one batch element per NeuronCore.

Per-core layout / algorithm (C=128 channels on SBUF partitions, N=4096 free):
  1. GroupNorm stats via two ACT passes (Square + Identity, both with
     accum_out row-sums), cross-partition group combine via tiny indicator
     matmuls on the PE.
  2. hn = a_c * x + b_c  (ACT + DVE, output bf16).
  3. Q, K in natural [c, n] layout (lhsT = host-pretransposed weights, bf16
     so the FWL fast-weight-load path engages); V^T in [n, c] tile-major
     layout (lhsT = hn tiles).  Only the rounds j=0 needs run before the
     main loop; the rest are woven into the early loop iterations so the
     QKV work rides the warm pipeline instead of a cold serial section.
  4. Main loop (2 halves x 32 j-tiles), software-pipelined so the PE never
     waits on its own iteration's exp: emit S(j) -> exp(j) -> O(j-1):
       S^T tile = K_j^T Q  (PE, bf16 in / f32 PSUM out),
       P^T = exp(S^T) (ACT, -> bf16),
       acc += P^T (DVE bf16 2x-mode denominator partials),
       O += V^T_j^T P^T (PE accumulate in PSUM, bf16 operands).
     No max-subtraction: logits are ~N(0,1) so exp is safe.
  5. Tail per half: denominators via a broadcast ones-matmul (every out
     partition gets the column sum of acc), reciprocal_approx_fast on DVE
     straight off PSUM (~18-bit), O_norm = O * recip
     (DVE), proj = w_proj^T O_norm (PE, borrowing the S PSUM tags),
     out = (x + b_eff) + proj, streamed to DRAM per 512-block.  The h0
     tail is interleaved into h1's main loop so only h1's tail is exposed.

Bias algebra: b_k is dropped entirely -- k_j = Wk hn_j + b_k adds q_i.b_k to
every logit of query i, a per-i constant that cancels exactly in softmax.
b_q folded into the Q PSUM->SBUF copy (DVE); b_v folded into
b_eff = b_proj + w_proj @ b_v (host precompute, exact).  The attention scale
C^-0.5 is folded into w_q/b_q on the host (exact reparameterization).

bf16 everywhere in attention: logit noise ~0.006 abs on N(0,1) logits and
0.4% weight noise post-softmax -- ~1e-3 relative on the final output vs the
2e-2 gate.
"""

import numpy as np

B, C, H, W = 8, 128, 64, 64
HW = H * W                      # 4096
GROUPS = 8
GSIZE = C // GROUPS             # 16
EPS = 1e-5
NJ = HW // 128                  # 32 j-tiles
IBLK = 512
NIB = HW // IBLK                # 8 i-blocks
NHALF = 2
HWID = HW // NHALF              # 2048
SCALE = float(C) ** -0.5

_CACHE = {}


def _dedup_ldweights(nc, mybir):
    """Remove back-to-back InstLdweights with identical weight APs (the
    TileContext exit splits every matmul into LDWEIGHTS+MATMUL; consecutive
    matmuls sharing a stationary reload it needlessly -- the PE array keeps
    the loaded weights).  Sync info from removed loads is merged into the
    following matmul, whose excess waits the later compile passes split into
    event semaphores."""
    removed = 0
    for f in nc.m.functions:
        for blk in f.blocks:
            insts = list(blk.instructions)
            cur_sig = None
            keep = []
            pending = None  # (waits, updates) from removed loads
            for i in insts:
                tn = type(i).__name__
                if tn == 'InstLdweights':
                    sig = repr(i.ins[0])
                    if sig == cur_sig:
                        si = i.sync_info
                        if si is not None and (len(si.on_wait) or len(si.on_update)):
                            w = list(si.on_wait)
                            u = list(si.on_update)
                            if pending:
                                w = pending[0] + w
                                u = pending[1] + u
                            pending = (w, u)
                        removed += 1
                        continue
                    cur_sig = sig
                elif tn == 'InstMatmult':
                    if getattr(i, 'is_transpose', False):
                        cur_sig = None
                    if pending is not None:
                        si = i.sync_info
                        ow = list(si.on_wait) if si else []
                        ou = list(si.on_update) if si else []
                        i.sync_info = mybir.SyncInfo(
                            on_wait=pending[0] + ow, on_update=pending[1] + ou
                        )
                        pending = None
                elif tn == 'InstMatmultMx':
                    cur_sig = None
                keep.append(i)
            if removed:
                assert pending is None, 'dangling sync with no following matmul'
                while len(blk.instructions):
                    blk.instructions.pop()
                for i in keep:
                    blk.instructions.append(i)
    return removed


def _build():
    from contextlib import ExitStack

    import concourse.bacc as bacc
    import concourse.tile as tile
    from concourse import mybir

    f32 = mybir.dt.float32
    bf16 = mybir.dt.bfloat16
    AF = mybir.ActivationFunctionType

    nc = bacc.Bacc("TRN2", target_bir_lowering=False, debug=False)

    x_in = nc.dram_tensor("x", [C, HW], f32, kind="ExternalInput")
    gamma_in = nc.dram_tensor("gamma", [C, 1], f32, kind="ExternalInput")
    beta_in = nc.dram_tensor("beta", [C, 1], f32, kind="ExternalInput")
    bq_in = nc.dram_tensor("bq", [C, 1], f32, kind="ExternalInput")
    beff_in = nc.dram_tensor("beff", [C, 1], f32, kind="ExternalInput")
    wq_in = nc.dram_tensor("wqT", [C, C], f32, kind="ExternalInput")
    wk_in = nc.dram_tensor("wkT", [C, C], f32, kind="ExternalInput")
    wv_in = nc.dram_tensor("wvT", [C, C], f32, kind="ExternalInput")
    wp_in = nc.dram_tensor("wpT", [C, C], f32, kind="ExternalInput")
    ig_in = nc.dram_tensor("ig", [C, GROUPS], f32, kind="ExternalInput")
    igt_in = nc.dram_tensor("igt", [GROUPS, C], f32, kind="ExternalInput")
    out_dram = nc.dram_tensor("out", [C, HW], f32, kind="ExternalOutput")

    with tile.TileContext(nc) as tc, ExitStack() as ctx, \
         nc.allow_low_precision(reason="bf16 attention pipeline; error "
                                "budget audited vs the 2e-2 gate"):
        const = ctx.enter_context(tc.tile_pool(name="const", bufs=1))
        big = ctx.enter_context(tc.tile_pool(name="big", bufs=1))
        stats = ctx.enter_context(tc.tile_pool(name="stats", bufs=1))
        ptpool = ctx.enter_context(tc.tile_pool(name="pt", bufs=3))
        stg = ctx.enter_context(tc.tile_pool(name="stage", bufs=2))

        # ---------------- load x on the sync+scalar queues; consts on the
        # vector queue so they don't serialize the x stream ----------------
        NCH = 8
        CHW = HW // NCH
        x_sb = big.tile([C, HW], f32, tag="x")
        for ch in range(NCH):
            sl = slice(ch * CHW, (ch + 1) * CHW)
            eng = nc.sync if ch % 2 == 0 else nc.scalar
            eng.dma_start(x_sb[:, sl], x_in[:, sl])

        def cload(t_in, shape, tag):
            t = const.tile(shape, f32, tag=tag)
            nc.sync.dma_start(t[:], t_in[:])
            return t

        gamma = cload(gamma_in, [C, 1], "c_gamma")
        beta = cload(beta_in, [C, 1], "c_beta")
        bq = cload(bq_in, [C, 1], "c_bq")
        beff = cload(beff_in, [C, 1], "c_beff")
        ig = cload(ig_in, [C, GROUPS], "c_ig")
        igt = cload(igt_in, [GROUPS, C], "c_igt")
        wq_f = cload(wq_in, [C, C], "c_wq_f")
        wk_f = cload(wk_in, [C, C], "c_wk_f")
        wv_f = cload(wv_in, [C, C], "c_wv_f")
        wp_f = cload(wp_in, [C, C], "c_wp_f")

        wq = const.tile([C, C], bf16)
        nc.vector.tensor_copy(wq[:], wq_f[:])
        wk = const.tile([C, C], bf16)
        nc.vector.tensor_copy(wk[:], wk_f[:])
        wv = const.tile([C, C], bf16)
        nc.vector.tensor_copy(wv[:], wv_f[:])
        wp = const.tile([C, C], bf16)
        nc.vector.tensor_copy(wp[:], wp_f[:])

        ones_bc = const.tile([C, C], bf16)
        nc.vector.memset(ones_bc[:], 1.0)

        eps_t = const.tile([GROUPS, 1], f32)
        nc.vector.memset(eps_t[:], EPS)
        magic_t = const.tile([GROUPS, 1], mybir.dt.uint32)
        nc.vector.memset(magic_t[:], 0x5F3759DF)
        c15_t = const.tile([GROUPS, 1], f32)
        nc.vector.memset(c15_t[:], 1.5)

        # ---------------- groupnorm stats (split across DVE and ACT) ----
        st2 = stats.tile([C, 2], f32)
        s2p = stats.tile([C, NCH], f32)
        s1p = stats.tile([C, NCH], f32)
        adum = stats.tile([C, CHW], f32)
        for ch in range(NCH):  # x^2 sums on ACT, x sums on DVE
            sl = slice(ch * CHW, (ch + 1) * CHW)
            nc.scalar.activation(
                adum[:], x_sb[:, sl], AF.Square, accum_out=s2p[:, ch:ch + 1]
            )
            nc.vector.reduce_sum(
                s1p[:, ch:ch + 1], x_sb[:, sl], axis=mybir.AxisListType.X
            )
        warm = stats.tile([GROUPS, 1], f32)
        nc.scalar.activation(warm[:], eps_t[:], AF.Exp)
        nc.vector.reduce_sum(st2[:, 1:2], s2p[:], axis=mybir.AxisListType.X)
        nc.vector.reduce_sum(st2[:, 0:1], s1p[:], axis=mybir.AxisListType.X)

        # PSUM layout for the whole kernel body: two S tiles (2 banks each,
        # independently released) + one O accumulator (4 banks). The QKV
        # rounds, groupnorm matmuls AND the projection tail all borrow the
        # S slots so there is no pool barrier anywhere.
        acc0 = big.tile([C, HWID], bf16, tag="acc0")
        acc1 = big.tile([C, HWID], bf16, tag="acc1")
        acch = [acc0, acc1]
        o_sb0 = big.tile([C, HWID], bf16, tag="o0")
        o_sb1 = big.tile([C, HWID], bf16, tag="o1")
        o_sbh = [o_sb0, o_sb1]
        out_sb = big.tile([C, HW], f32, tag="scratch")
        rbc0 = big.tile([C, HWID], f32, tag="rbc0")
        rbc1 = big.tile([C, HWID], f32, tag="rbc1")
        rbch = [rbc0, rbc1]
        HQ = HWID // 2  # 1024
        with tc.tile_pool(name="ps_s", bufs=1, space="PSUM") as ps_s, \
             tc.tile_pool(name="ps_o", bufs=1, space="PSUM") as ps_o:
            gs_ps = ps_s.tile([GROUPS, 2], f32, tag="s0")
            nc.tensor.matmul(gs_ps[:], ig[:], st2[:], start=True, stop=True)
            gstats = stats.tile([GROUPS, 2], f32)
            nc.vector.tensor_copy(gstats[:], gs_ps[:])
            inv_n = 1.0 / float(GSIZE * HW)
            gmean = stats.tile([GROUPS, 1], f32)
            nc.vector.tensor_scalar_mul(gmean[:], gstats[:, 0:1], inv_n)
            gm2 = stats.tile([GROUPS, 1], f32)
            nc.vector.tensor_scalar_mul(gm2[:], gstats[:, 1:2], inv_n)
            gmsq = stats.tile([GROUPS, 1], f32)
            nc.vector.tensor_mul(gmsq[:], gmean[:], gmean[:])
            gvar = stats.tile([GROUPS, 1], f32)
            nc.vector.tensor_sub(gvar[:], gm2[:], gmsq[:])
            gve = stats.tile([GROUPS, 1], f32)
            nc.vector.tensor_scalar(
                gve[:], gvar[:], eps_t[:], None, mybir.AluOpType.add
            )
            # rstd = rsqrt(var+eps): quake initial guess + Newton steps (DVE
            # only -- ACT Sqrt/Ln would each force a ~1.3us table-set swap)
            u32 = mybir.dt.uint32
            gu = stats.tile([GROUPS, 1], u32)
            nc.vector.tensor_scalar(
                gu[:], gve[:].bitcast(u32), 1, None,
                mybir.AluOpType.logical_shift_right,
            )
            nc.vector.tensor_sub(gu[:], magic_t[:], gu[:])
            gy = stats.tile([GROUPS, 1], f32)
            nc.vector.tensor_copy(gy[:], gu[:].bitcast(f32))
            gh = stats.tile([GROUPS, 1], f32)
            nc.vector.tensor_scalar_mul(gh[:], gve[:], 0.5)
            gt = stats.tile([GROUPS, 1], f32)
            for _ in range(2):
                nc.vector.tensor_mul(gt[:], gy[:], gy[:])
                nc.vector.tensor_mul(gt[:], gt[:], gh[:])
                nc.vector.tensor_sub(gt[:], c15_t[:], gt[:])
                nc.vector.tensor_mul(gy[:], gy[:], gt[:])
            gmr = stats.tile([GROUPS, 2], f32)
            nc.vector.tensor_copy(gmr[:, 1:2], gy[:])
            nc.vector.tensor_copy(gmr[:, 0:1], gmean[:])

            bc_ps = ps_s.tile([C, 2], f32, tag="s0")
            nc.tensor.matmul(bc_ps[:], igt[:], gmr[:], start=True, stop=True)
            # warm the PE HAM clock across the a_c/b_c scalar chain and the
            # first hn pass, so the QKV stream starts at full clock (dummy
            # f32 matmuls, results never read; slot reused later)
            wmt = ps_s.tile([C, HQ], f32, tag="s1")
            for _ in range(5):
                nc.tensor.matmul(
                    wmt[:, 0:IBLK], wq_f[:], x_sb[:, 0:IBLK],
                    start=True, stop=True,
                )
            a_c = stats.tile([C, 1], f32)
            b_c = stats.tile([C, 1], f32)
            tmc = stats.tile([C, 1], f32)
            nc.vector.tensor_scalar_mul(a_c[:], gamma[:], bc_ps[:, 1:2])
            nc.vector.tensor_scalar_mul(tmc[:], a_c[:], bc_ps[:, 0:1])
            nc.vector.tensor_sub(b_c[:], beta[:], tmc[:])

            hn = big.tile([C, HW], bf16, tag="hn")
            q_r = big.tile([C, HW], bf16, tag="q")
            k_r = big.tile([C, HW], bf16, tag="k")
            vt = big.tile([C, NJ, C], bf16, tag="vt")

            def emit_hn(h, engine):
                hs = slice(h * HWID, (h + 1) * HWID)
                if engine == "act":
                    nc.scalar.activation(
                        hn[:, hs], x_sb[:, hs], AF.Identity, bias=b_c[:], scale=a_c[:]
                    )
                else:
                    nc.vector.tensor_scalar(
                        hn[:, hs], x_sb[:, hs], a_c[:], b_c[:],
                        mybir.AluOpType.mult, mybir.AluOpType.add,
                    )

            def emit_k_round(h, r):  # r in 0..1, [C, HQ] rounds
                kp = ps_s.tile([C, HQ], f32, tag=f"s{r % 2}")
                for kk in range(2):
                    off = h * HWID + r * HQ + kk * IBLK
                    nc.tensor.matmul(
                        kp[:, kk * IBLK:(kk + 1) * IBLK], wk[:],
                        hn[:, off:off + IBLK], start=True, stop=True,
                    )
                # b_k dropped: a per-query constant in the logits, cancels in
                # softmax exactly.
                nc.vector.tensor_copy(
                    k_r[:, h * HWID + r * HQ:h * HWID + (r + 1) * HQ], kp[:]
                )

            def emit_q_round(h, r):
                qp = ps_s.tile([C, HQ], f32, tag=f"s{r % 2}")
                for kk in range(2):
                    off = h * HWID + r * HQ + kk * IBLK
                    nc.tensor.matmul(
                        qp[:, kk * IBLK:(kk + 1) * IBLK], wq[:],
                        hn[:, off:off + IBLK], start=True, stop=True,
                    )
                nc.vector.tensor_scalar(
                    q_r[:, h * HWID + r * HQ:h * HWID + (r + 1) * HQ], qp[:],
                    bq[:], None, mybir.AluOpType.add,
                )

            def emit_v_round(h, r):  # r in 0..3, 4 n-tiles per round
                vp = ps_s.tile([C, 4, C], f32, tag=f"s{r % 2}")
                for t in range(4):
                    nt = h * 16 + r * 4 + t
                    nc.tensor.matmul(
                        vp[:, t, :], hn[:, nt * 128:(nt + 1) * 128], wv[:],
                        start=True, stop=True,
                    )
                tsl = slice(h * 16 + r * 4, h * 16 + (r + 1) * 4)
                nc.vector.tensor_copy(vt[:, tsl, :], vp[:])

            # minimal preamble: only what j=0..4 of the h0 loop needs.  The
            # remaining K/V/Q rounds are woven into the early loop
            # iterations (each round is emitted well before the first
            # iteration that consumes its output), so the QKV work rides
            # the warm main-loop pipeline instead of a cold serial section.
            emit_hn(0, "act")
            for r in range(2):
                emit_k_round(0, r)
            for r in range(2):
                emit_q_round(0, r)
            emit_hn(1, "dve")
            emit_v_round(0, 0)

            def pre_step(h, j):
                if h != 0:
                    return
                if j == 1:
                    emit_v_round(0, 1)      # vt 4-7, first used at j=5
                elif j == 2:
                    emit_v_round(0, 2)      # vt 8-11 @ j=9
                elif j == 3:
                    emit_v_round(0, 3)      # vt 12-15 @ j=13
                elif j == 4:
                    emit_k_round(1, 0)      # k_r j-tiles 16-23 @ j=16
                elif j == 5:
                    emit_k_round(1, 1)      # k_r j-tiles 24-31 @ j=24
                elif j == 6:
                    emit_v_round(1, 0)      # vt 16-19 @ j=17
                elif j == 7:
                    emit_v_round(1, 1)      # vt 20-23 @ j=21
                elif j == 8:
                    emit_v_round(1, 2)      # vt 24-27 @ j=25
                elif j == 9:
                    emit_v_round(1, 3)      # vt 28-31 @ j=29
                elif j == 26:
                    emit_q_round(1, 0)      # q_r h1 first half, for h1 j=0
                elif j == 28:
                    emit_q_round(1, 1)      # q_r h1 second half

            # ------------- denominator + projection tail emitters --------
            def emit_denom(h, q):
                # cross-partition sum of one [C, HQ] quarter of acc via a
                # broadcast ones-matmul (every out partition = column sum),
                # then 1/x straight off PSUM on DVE (~18 bits correct)
                qs = slice(q * HQ, (q + 1) * HQ)
                dt_ = ps_s.tile([C, HQ], f32, tag=f"s{q % 2}")
                for kk in range(2):
                    ds = slice(q * HQ + kk * IBLK, q * HQ + (kk + 1) * IBLK)
                    nc.tensor.matmul(
                        dt_[:, kk * IBLK:(kk + 1) * IBLK], ones_bc[:],
                        acch[h][:, ds], start=True, stop=True,
                    )
                nc.vector.reciprocal_approx_fast(rbch[h][:, qs], dt_[:])

            def emit_proj(ib, h, base):
                # ib is the global 512-block index; base = offset in-half
                sl = slice(ib * IBLK, (ib + 1) * IBLK)
                o_nrm = stg.tile([C, IBLK], bf16, tag="onrm")
                nc.vector.tensor_mul(
                    o_nrm[:], o_sbh[h][:, base:base + IBLK],
                    rbch[h][:, base:base + IBLK],
                )
                bt = ps_s.tile([C, HQ], f32, tag=f"s{ib % 2}")
                pp = bt[:, 0:IBLK]
                nc.tensor.matmul(pp, wp[:], o_nrm[:], start=True, stop=True)
                # out = (proj + b_eff) + x in a single DVE pass
                nc.vector.scalar_tensor_tensor(
                    out_sb[:, sl], pp, beff[:], x_sb[:, sl],
                    mybir.AluOpType.add, mybir.AluOpType.add,
                )
                nc.sync.dma_start(out_dram[:, sl], out_sb[:, sl])

            # ---------------- main attention loop ----------------
            # Software-pipelined: iteration j emits S(j) matmuls, exp(j),
            # then O(j-1), so the PE never stalls on its own iteration's exp.
            def tail_step(h, j):
                # interleave h0's tail into h1's loop (emitted before the
                # acc add of iteration j so the DVE reaches the PSUM reads
                # quickly and releases the borrowed S tag)
                if h != 1:
                    return
                if j == 3:
                    emit_denom(0, 0)
                elif j == 5:
                    emit_denom(0, 1)
                elif j in (9, 13, 17, 21):
                    ib = (j - 9) // 4
                    emit_proj(ib, 0, ib * IBLK)

            for h in range(NHALF):
                op = ps_o.tile([C, HWID], f32, tag="o_ps")
                prev = None
                for j in range(NJ):
                    pre_step(h, j)
                    pt = ptpool.tile([C, HWID], bf16)
                    for half in range(2):
                        sp = ps_s.tile([C, HQ], f32, tag=f"s{half}")
                        for kk in range(2):
                            qoff = h * HWID + half * HQ + kk * IBLK
                            nc.tensor.matmul(
                                sp[:, kk * IBLK:(kk + 1) * IBLK],
                                k_r[:, j * 128:(j + 1) * 128],
                                q_r[:, qoff:qoff + IBLK],
                                start=True, stop=True,
                            )
                        nc.scalar.activation(
                            pt[:, half * HQ:(half + 1) * HQ], sp[:], AF.Exp
                        )
                    if prev is not None:
                        for kk in range(4):
                            sl = slice(kk * IBLK, (kk + 1) * IBLK)
                            nc.tensor.matmul(
                                op[:, sl], vt[:, j - 1, :], prev[:, sl],
                                start=(j == 1), stop=False,
                            )
                    tail_step(h, j)
                    if j == 0:
                        nc.vector.tensor_copy(acch[h][:], pt[:])
                    else:
                        nc.vector.tensor_add(acch[h][:], acch[h][:], pt[:])
                    prev = pt
                for kk in range(4):
                    sl = slice(kk * IBLK, (kk + 1) * IBLK)
                    nc.tensor.matmul(
                        op[:, sl], vt[:, NJ - 1, :], prev[:, sl],
                        start=False, stop=True,
                    )
                nc.vector.tensor_copy(o_sbh[h][:], op[:])

            # h1 tail (h0's was interleaved above)
            emit_denom(1, 0)
            emit_denom(1, 1)
            for ib in range(4, 8):
                emit_proj(ib, 1, (ib - 4) * IBLK)

    _dedup_ldweights(nc, __import__('concourse.mybir', fromlist=['mybir']))
    nc.compile()
    return nc


def _get_nc():
    if "nc" not in _CACHE:
        _CACHE["nc"] = _build()
    return _CACHE["nc"]


def _prep_inputs(x, gamma, beta, w_qkv, b_qkv, w_proj, b_proj):
    x = np.ascontiguousarray(x, dtype=np.float32)
    w_qkv = np.asarray(w_qkv, dtype=np.float32)
    b_qkv = np.asarray(b_qkv, dtype=np.float32)
    w_proj = np.asarray(w_proj, dtype=np.float32)
    b_proj = np.asarray(b_proj, dtype=np.float32)

    wq = w_qkv[0:C, :]
    wk = w_qkv[C:2 * C, :]
    wv = w_qkv[2 * C:3 * C, :]
    bqv = b_qkv[0:C]
    bvv = b_qkv[2 * C:3 * C]

    wqT = np.ascontiguousarray((wq * SCALE).T)
    wkT = np.ascontiguousarray(wk.T)
    wvT = np.ascontiguousarray(wv.T)
    wpT = np.ascontiguousarray(w_proj.T)
    beff = (b_proj + w_proj @ bvv).astype(np.float32)

    ig = np.zeros((C, GROUPS), np.float32)
    ig[np.arange(C), np.arange(C) // GSIZE] = 1.0
    igt = np.ascontiguousarray(ig.T)

    common = {
        "gamma": np.asarray(gamma, np.float32).reshape(C, 1),
        "beta": np.asarray(beta, np.float32).reshape(C, 1),
        "bq": (bqv * SCALE).reshape(C, 1),
        "beff": beff.reshape(C, 1),
        "wqT": wqT,
        "wkT": wkT,
        "wvT": wvT,
        "wpT": wpT,
        "ig": ig,
        "igt": igt,
    }
    in_maps = []
    for b in range(B):
        m = dict(common)
        m["x"] = np.ascontiguousarray(x[b].reshape(C, HW))
        in_maps.append(m)
    return in_maps


def kernel(x, gamma, beta, w_qkv, b_qkv, w_proj, b_proj):
    from concourse.bass_utils import run_bass_kernel_spmd

    nc = _get_nc()
    in_maps = _prep_inputs(x, gamma, beta, w_qkv, b_qkv, w_proj, b_proj)
    res = run_bass_kernel_spmd(nc, in_maps, list(range(B)))
    out = np.stack([res.results[b]["out"] for b in range(B)], axis=0)
    return out.reshape(B, C, H, W).astype(np.float32)


# revision 20
# speedup vs baseline: 1.3792x; 1.0150x over previous
"""Trainium2 Bass kernel for nn_AttentionBlock (GroupNorm + 1x1-conv QKV +
full self-attention over N=HW=4096 + output projection + residual).

Distribution: data-parallel over batch B=8, one batch element per NeuronCore.

Per-core layout / algorithm (C=128 channels on SBUF partitions, N=4096 free):
  1. GroupNorm stats via two ACT passes (Square + Identity, both with
     accum_out row-sums), cross-partition group combine via tiny indicator
     matmuls on the PE.
  2. hn = a_c * x + b_c  (ACT + DVE, output bf16).
  3. Q, K in natural [c, n] layout (lhsT = host-pretransposed weights, bf16
     so the FWL fast-weight-load path engages); V^T in [n, c] tile-major
     layout (lhsT = hn tiles).  Only the rounds j=0 needs run before the
     main loop; the rest are woven into the early loop iterations so the
     QKV work rides the warm pipeline instead of a cold serial section.
  4. Main loop (2 halves x 32 j-tiles), software-pipelined so the PE never
     waits on its own iteration's exp: emit S(j) -> exp(j) -> O(j-1):
       S^T tile = K_j^T Q  (PE, bf16 in / f32 PSUM out),
       P^T = exp(S^T) (ACT, -> bf16),
       acc += P^T (DVE bf16 2x-mode denominator partials),
       O += V^T_j^T P^T (PE accumulate in PSUM, bf16 operands).
     No max-subtraction: logits are ~N(0,1) so exp is safe.
  5. Tail per half: denominators via a broadcast ones-matmul (every out
     partition gets the column sum of acc), reciprocal_approx_fast on DVE
     straight off PSUM (~18-bit), O_norm = O * recip
     (DVE), proj = w_proj^T O_norm (PE, borrowing the S PSUM tags),
     out = (x + b_eff) + proj, streamed to DRAM per 512-block.  The h0
     tail is interleaved into h1's main loop so only h1's tail is exposed.

Bias algebra: b_k is dropped entirely -- k_j = Wk hn_j + b_k adds q_i.b_k to
every logit of query i, a per-i constant that cancels exactly in softmax.
b_q folded into the Q PSUM->SBUF copy (DVE); b_v folded into
b_eff = b_proj + w_proj @ b_v (host precompute, exact).  The attention scale
C^-0.5 is folded into w_q/b_q on the host (exact reparameterization).

bf16 everywhere in attention: logit noise ~0.006 abs on N(0,1) logits and
0.4% weight noise post-softmax -- ~1e-3 relative on the final output vs the
2e-2 gate.
"""

import numpy as np

B, C, H, W = 8, 128, 64, 64
HW = H * W                      # 4096
GROUPS = 8
GSIZE = C // GROUPS             # 16
EPS = 1e-5
NJ = HW // 128                  # 32 j-tiles
IBLK = 512
NIB = HW // IBLK                # 8 i-blocks
NHALF = 2
HWID = HW // NHALF              # 2048
SCALE = float(C) ** -0.5

_CACHE = {}


def _dedup_ldweights(nc, mybir):
    """Remove back-to-back InstLdweights with identical weight APs (the
    TileContext exit splits every matmul into LDWEIGHTS+MATMUL; consecutive
    matmuls sharing a stationary reload it needlessly -- the PE array keeps
    the loaded weights).  Sync info from removed loads is merged into the
    following matmul, whose excess waits the later compile passes split into
    event semaphores."""
    removed = 0
    for f in nc.m.functions:
        for blk in f.blocks:
            insts = list(blk.instructions)
            cur_sig = None
            keep = []
            pending = None  # (waits, updates) from removed loads
            for i in insts:
                tn = type(i).__name__
                if tn == 'InstLdweights':
                    sig = repr(i.ins[0])
                    if sig == cur_sig:
                        si = i.sync_info
                        if si is not None and (len(si.on_wait) or len(si.on_update)):
                            w = list(si.on_wait)
                            u = list(si.on_update)
                            if pending:
                                w = pending[0] + w
                                u = pending[1] + u
                            pending = (w, u)
                        removed += 1
                        continue
                    cur_sig = sig
                elif tn == 'InstMatmult':
                    if getattr(i, 'is_transpose', False):
                        cur_sig = None
                    if pending is not None:
                        si = i.sync_info
                        ow = list(si.on_wait) if si else []
                        ou = list(si.on_update) if si else []
                        i.sync_info = mybir.SyncInfo(
                            on_wait=pending[0] + ow, on_update=pending[1] + ou
                        )
                        pending = None
                elif tn == 'InstMatmultMx':
                    cur_sig = None
                keep.append(i)
            if removed:
                assert pending is None, 'dangling sync with no following matmul'
                while len(blk.instructions):
                    blk.instructions.pop()
                for i in keep:
                    blk.instructions.append(i)
    return removed


def _build():
    from contextlib import ExitStack

    import concourse.bacc as bacc
    import concourse.tile as tile
    from concourse import mybir

    f32 = mybir.dt.float32
    bf16 = mybir.dt.bfloat16
    AF = mybir.ActivationFunctionType

    nc = bacc.Bacc("TRN2", target_bir_lowering=False, debug=False)

    x_in = nc.dram_tensor("x", [C, HW], f32, kind="ExternalInput")
    gamma_in = nc.dram_tensor("gamma", [C, 1], f32, kind="ExternalInput")
    beta_in = nc.dram_tensor("beta", [C, 1], f32, kind="ExternalInput")
    bq_in = nc.dram_tensor("bq", [C, 1], f32, kind="ExternalInput")
    beff_in = nc.dram_tensor("beff", [C, 1], f32, kind="ExternalInput")
    wq_in = nc.dram_tensor("wqT", [C, C], f32, kind="ExternalInput")
    wk_in = nc.dram_tensor("wkT", [C, C], f32, kind="ExternalInput")
    wv_in = nc.dram_tensor("wvT", [C, C], f32, kind="ExternalInput")
    wp_in = nc.dram_tensor("wpT", [C, C], f32, kind="ExternalInput")
    ig_in = nc.dram_tensor("ig", [C, GROUPS], f32, kind="ExternalInput")
    igt_in = nc.dram_tensor("igt", [GROUPS, C], f32, kind="ExternalInput")
    out_dram = nc.dram_tensor("out", [C, HW], f32, kind="ExternalOutput")

    with tile.TileContext(nc) as tc, ExitStack() as ctx, \
         nc.allow_low_precision(reason="bf16 attention pipeline; error "
                                "budget audited vs the 2e-2 gate"):
        const = ctx.enter_context(tc.tile_pool(name="const", bufs=1))
        big = ctx.enter_context(tc.tile_pool(name="big", bufs=1))
        stats = ctx.enter_context(tc.tile_pool(name="stats", bufs=1))
        ptpool = ctx.enter_context(tc.tile_pool(name="pt", bufs=3))
        stg = ctx.enter_context(tc.tile_pool(name="stage", bufs=2))

        # ---------------- load x on the sync+scalar queues; consts on the
        # vector queue so they don't serialize the x stream ----------------
        NCH = 8
        CHW = HW // NCH
        x_sb = big.tile([C, HW], f32, tag="x")
        for ch in range(NCH):
            sl = slice(ch * CHW, (ch + 1) * CHW)
            eng = nc.sync if ch % 2 == 0 else nc.scalar
            eng.dma_start(x_sb[:, sl], x_in[:, sl])

        def cload(t_in, shape, tag):
            t = const.tile(shape, f32, tag=tag)
            nc.sync.dma_start(t[:], t_in[:])
            return t

        gamma = cload(gamma_in, [C, 1], "c_gamma")
        beta = cload(beta_in, [C, 1], "c_beta")
        bq = cload(bq_in, [C, 1], "c_bq")
        beff = cload(beff_in, [C, 1], "c_beff")
        ig = cload(ig_in, [C, GROUPS], "c_ig")
        igt = cload(igt_in, [GROUPS, C], "c_igt")
        wq_f = cload(wq_in, [C, C], "c_wq_f")
        wk_f = cload(wk_in, [C, C], "c_wk_f")
        wv_f = cload(wv_in, [C, C], "c_wv_f")
        wp_f = cload(wp_in, [C, C], "c_wp_f")

        wq = const.tile([C, C], bf16)
        nc.vector.tensor_copy(wq[:], wq_f[:])
        wk = const.tile([C, C], bf16)
        nc.vector.tensor_copy(wk[:], wk_f[:])
        wv = const.tile([C, C], bf16)
        nc.vector.tensor_copy(wv[:], wv_f[:])
        wp = const.tile([C, C], bf16)
        nc.vector.tensor_copy(wp[:], wp_f[:])

        ones_bc = const.tile([C, C], bf16)
        nc.vector.memset(ones_bc[:], 1.0)

        eps_t = const.tile([GROUPS, 1], f32)
        nc.vector.memset(eps_t[:], EPS)
        magic_t = const.tile([GROUPS, 1], mybir.dt.uint32)
        nc.vector.memset(magic_t[:], 0x5F3759DF)
        c15_t = const.tile([GROUPS, 1], f32)
        nc.vector.memset(c15_t[:], 1.5)

        # ---------------- groupnorm stats (split across DVE and ACT) ----
        st2 = stats.tile([C, 2], f32)
        s2p = stats.tile([C, NCH], f32)
        s1p = stats.tile([C, NCH], f32)
        adum = stats.tile([C, CHW], f32)
        for ch in range(NCH):  # x^2 sums on ACT, x sums on DVE
            sl = slice(ch * CHW, (ch + 1) * CHW)
            nc.scalar.activation(
                adum[:], x_sb[:, sl], AF.Square, accum_out=s2p[:, ch:ch + 1]
            )
            nc.vector.reduce_sum(
                s1p[:, ch:ch + 1], x_sb[:, sl], axis=mybir.AxisListType.X
            )
        warm = stats.tile([GROUPS, 1], f32)
        nc.scalar.activation(warm[:], eps_t[:], AF.Exp)
        nc.vector.reduce_sum(st2[:, 1:2], s2p[:], axis=mybir.AxisListType.X)
        nc.vector.reduce_sum(st2[:, 0:1], s1p[:], axis=mybir.AxisListType.X)

        # PSUM layout for the whole kernel body: two S tiles (2 banks each,
        # independently released) + one O accumulator (4 banks). The QKV
        # rounds, groupnorm matmuls AND the projection tail all borrow the
        # S slots so there is no pool barrier anywhere.
        acc0 = big.tile([C, HWID], bf16, tag="acc0")
        acc1 = big.tile([C, HWID], bf16, tag="acc1")
        acch = [acc0, acc1]
        o_sb0 = big.tile([C, HWID], bf16, tag="o0")
        o_sb1 = big.tile([C, HWID], bf16, tag="o1")
        o_sbh = [o_sb0, o_sb1]
        out_sb = big.tile([C, HW], f32, tag="scratch")
        rbc0 = big.tile([C, HWID], f32, tag="rbc0")
        rbc1 = big.tile([C, HWID], f32, tag="rbc1")
        rbch = [rbc0, rbc1]
        HQ = HWID // 2  # 1024
        with tc.tile_pool(name="ps_s", bufs=1, space="PSUM") as ps_s, \
             tc.tile_pool(name="ps_o", bufs=1, space="PSUM") as ps_o:
            gs_ps = ps_s.tile([GROUPS, 2], f32, tag="s0")
            nc.tensor.matmul(gs_ps[:], ig[:], st2[:], start=True, stop=True)
            gstats = stats.tile([GROUPS, 2], f32)
            nc.vector.tensor_copy(gstats[:], gs_ps[:])
            inv_n = 1.0 / float(GSIZE * HW)
            gmean = stats.tile([GROUPS, 1], f32)
            nc.vector.tensor_scalar_mul(gmean[:], gstats[:, 0:1], inv_n)
            gm2 = stats.tile([GROUPS, 1], f32)
            nc.vector.tensor_scalar_mul(gm2[:], gstats[:, 1:2], inv_n)
            gmsq = stats.tile([GROUPS, 1], f32)
            nc.vector.tensor_mul(gmsq[:], gmean[:], gmean[:])
            gvar = stats.tile([GROUPS, 1], f32)
            nc.vector.tensor_sub(gvar[:], gm2[:], gmsq[:])
            gve = stats.tile([GROUPS, 1], f32)
            nc.vector.tensor_scalar(
                gve[:], gvar[:], eps_t[:], None, mybir.AluOpType.add
            )
            # rstd = rsqrt(var+eps): quake initial guess + Newton steps (DVE
            # only -- ACT Sqrt/Ln would each force a ~1.3us table-set swap)
            u32 = mybir.dt.uint32
            gu = stats.tile([GROUPS, 1], u32)
            nc.vector.tensor_scalar(
                gu[:], gve[:].bitcast(u32), 1, None,
                mybir.AluOpType.logical_shift_right,
            )
            nc.vector.tensor_sub(gu[:], magic_t[:], gu[:])
            gy = stats.tile([GROUPS, 1], f32)
            nc.vector.tensor_copy(gy[:], gu[:].bitcast(f32))
            gh = stats.tile([GROUPS, 1], f32)
            nc.vector.tensor_scalar_mul(gh[:], gve[:], 0.5)
            gt = stats.tile([GROUPS, 1], f32)
            for _ in range(2):
                nc.vector.tensor_mul(gt[:], gy[:], gy[:])
                nc.vector.tensor_mul(gt[:], gt[:], gh[:])
                nc.vector.tensor_sub(gt[:], c15_t[:], gt[:])
                nc.vector.tensor_mul(gy[:], gy[:], gt[:])
            gmr = stats.tile([GROUPS, 2], f32)
            nc.vector.tensor_copy(gmr[:, 1:2], gy[:])
            nc.vector.tensor_copy(gmr[:, 0:1], gmean[:])

            bc_ps = ps_s.tile([C, 2], f32, tag="s0")
            nc.tensor.matmul(bc_ps[:], igt[:], gmr[:], start=True, stop=True)
            # warm the PE HAM clock across the a_c/b_c scalar chain and the
            # first hn pass, so the QKV stream starts at full clock (dummy
            # f32 matmuls, results never read; slot reused later)
            wmt = ps_s.tile([C, HQ], f32, tag="s1")
            for _ in range(5):
                nc.tensor.matmul(
                    wmt[:, 0:IBLK], wq_f[:], x_sb[:, 0:IBLK],
                    start=True, stop=True,
                )
            a_c = stats.tile([C, 1], f32)
            b_c = stats.tile([C, 1], f32)
            tmc = stats.tile([C, 1], f32)
            nc.vector.tensor_scalar_mul(a_c[:], gamma[:], bc_ps[:, 1:2])
            nc.vector.tensor_scalar_mul(tmc[:], a_c[:], bc_ps[:, 0:1])
            nc.vector.tensor_sub(b_c[:], beta[:], tmc[:])

            hn = big.tile([C, HW], bf16, tag="hn")
            q_r = big.tile([C, HW], bf16, tag="q")
            k_r = big.tile([C, HW], bf16, tag="k")
            vt = big.tile([C, NJ, C], bf16, tag="vt")

            def emit_hn(h, engine):
                hs = slice(h * HWID, (h + 1) * HWID)
                if engine == "act":
                    nc.scalar.activation(
                        hn[:, hs], x_sb[:, hs], AF.Identity, bias=b_c[:], scale=a_c[:]
                    )
                else:
                    nc.vector.tensor_scalar(
                        hn[:, hs], x_sb[:, hs], a_c[:], b_c[:],
                        mybir.AluOpType.mult, mybir.AluOpType.add,
                    )

            def emit_k_round(h, r):  # r in 0..1, [C, HQ] rounds
                kp = ps_s.tile([C, HQ], f32, tag=f"s{r % 2}")
                for kk in range(2):
                    off = h * HWID + r * HQ + kk * IBLK
                    nc.tensor.matmul(
                        kp[:, kk * IBLK:(kk + 1) * IBLK], wk[:],
                        hn[:, off:off + IBLK], start=True, stop=True,
                    )
                # b_k dropped: a per-query constant in the logits, cancels in
                # softmax exactly.
                nc.vector.tensor_copy(
                    k_r[:, h * HWID + r * HQ:h * HWID + (r + 1) * HQ], kp[:]
                )

            def emit_q_round(h, r):
                qp = ps_s.tile([C, HQ], f32, tag=f"s{r % 2}")
                for kk in range(2):
                    off = h * HWID + r * HQ + kk * IBLK
                    nc.tensor.matmul(
                        qp[:, kk * IBLK:(kk + 1) * IBLK], wq[:],
                        hn[:, off:off + IBLK], start=True, stop=True,
                    )
                nc.vector.tensor_scalar(
                    q_r[:, h * HWID + r * HQ:h * HWID + (r + 1) * HQ], qp[:],
                    bq[:], None, mybir.AluOpType.add,
                )

            def emit_v_round(h, r):  # r in 0..3, 4 n-tiles per round
                vp = ps_s.tile([C, 4, C], f32, tag=f"s{r % 2}")
                for t in range(4):
                    nt = h * 16 + r * 4 + t
                    nc.tensor.matmul(
                        vp[:, t, :], hn[:, nt * 128:(nt + 1) * 128], wv[:],
                        start=True, stop=True,
                    )
                tsl = slice(h * 16 + r * 4, h * 16 + (r + 1) * 4)
                nc.vector.tensor_copy(vt[:, tsl, :], vp[:])

            # minimal preamble: only what j=0..4 of the h0 loop needs.  The
            # remaining K/V/Q rounds are woven into the early loop
            # iterations (each round is emitted well before the first
            # iteration that consumes its output), so the QKV work rides
            # the warm main-loop pipeline instead of a cold serial section.
            emit_hn(0, "act")
            for r in range(2):
                emit_k_round(0, r)
            for r in range(2):
                emit_q_round(0, r)
            emit_hn(1, "dve")
            emit_v_round(0, 0)

            def pre_step(h, j):
                if h != 0:
                    return
                if j == 1:
                    emit_v_round(0, 1)      # vt 4-7, first used at j=5
                elif j == 2:
                    emit_v_round(0, 2)      # vt 8-11 @ j=9
                elif j == 3:
                    emit_v_round(0, 3)      # vt 12-15 @ j=13
                elif j == 4:
                    emit_k_round(1, 0)      # k_r j-tiles 16-23 @ j=16
                elif j == 5:
                    emit_k_round(1, 1)      # k_r j-tiles 24-31 @ j=24
                elif j == 6:
                    emit_v_round(1, 0)      # vt 16-19 @ j=17
                elif j == 7:
                    emit_v_round(1, 1)      # vt 20-23 @ j=21
                elif j == 8:
                    emit_v_round(1, 2)      # vt 24-27 @ j=25
                elif j == 9:
                    emit_v_round(1, 3)      # vt 28-31 @ j=29
                elif j == 26:
                    emit_q_round(1, 0)      # q_r h1 first half, for h1 j=0
                elif j == 28:
                    emit_q_round(1, 1)      # q_r h1 second half

            # ------------- denominator + projection tail emitters --------
            def emit_denom(h, q):
                # cross-partition sum of one [C, HQ] quarter of acc via a
                # broadcast ones-matmul (every out partition = column sum),
                # then 1/x straight off PSUM on DVE (~18 bits correct)
                qs = slice(q * HQ, (q + 1) * HQ)
                dt_ = ps_s.tile([C, HQ], f32, tag=f"s{q % 2}")
                for kk in range(2):
                    ds = slice(q * HQ + kk * IBLK, q * HQ + (kk + 1) * IBLK)
                    nc.tensor.matmul(
                        dt_[:, kk * IBLK:(kk + 1) * IBLK], ones_bc[:],
                        acch[h][:, ds], start=True, stop=True,
                    )
                nc.vector.reciprocal_approx_fast(rbch[h][:, qs], dt_[:])

            def emit_proj(ib, h, base):
                # ib is the global 512-block index; base = offset in-half
                sl = slice(ib * IBLK, (ib + 1) * IBLK)
                o_nrm = stg.tile([C, IBLK], bf16, tag="onrm")
                nc.vector.tensor_mul(
                    o_nrm[:], o_sbh[h][:, base:base + IBLK],
                    rbch[h][:, base:base + IBLK],
                )
                bt = ps_s.tile([C, HQ], f32, tag=f"s{ib % 2}")
                pp = bt[:, 0:IBLK]
                nc.tensor.matmul(pp, wp[:], o_nrm[:], start=True, stop=True)
                # out = (proj + b_eff) + x in a single DVE pass
                nc.vector.scalar_tensor_tensor(
                    out_sb[:, sl], pp, beff[:], x_sb[:, sl],
                    mybir.AluOpType.add, mybir.AluOpType.add,
                )
                nc.sync.dma_start(out_dram[:, sl], out_sb[:, sl])

            # ---------------- main attention loop ----------------
            # Software-pipelined: iteration j emits S(j) matmuls, exp(j),
            # then O(j-1), so the PE never stalls on its own iteration's exp.
            def tail_step(h, j):
                # interleave h0's tail into h1's loop (emitted before the
                # acc add of iteration j so the DVE reaches the PSUM reads
                # quickly and releases the borrowed S tag)
                if h != 1:
                    return
                if j == 3:
                    emit_denom(0, 0)
                elif j == 5:
                    emit_denom(0, 1)
                elif j in (9, 13, 17, 21):
                    ib = (j - 9) // 4
                    emit_proj(ib, 0, ib * IBLK)

            for h in range(NHALF):
                op = ps_o.tile([C, HWID], f32, tag="o_ps")
                prev = None
                for j in range(NJ):
                    pre_step(h, j)
                    pt = ptpool.tile([C, HWID], bf16)
                    for half in range(2):
                        sp = ps_s.tile([C, HQ], f32, tag=f"s{half}")
                        for kk in range(2):
                            qoff = h * HWID + half * HQ + kk * IBLK
                            nc.tensor.matmul(
                                sp[:, kk * IBLK:(kk + 1) * IBLK],
                                k_r[:, j * 128:(j + 1) * 128],
                                q_r[:, qoff:qoff + IBLK],
                                start=True, stop=True,
                            )
                        nc.scalar.activation(
                            pt[:, half * HQ:(half + 1) * HQ], sp[:], AF.Exp
                        )
                    if prev is not None:
                        for kk in range(4):
                            sl = slice(kk * IBLK, (kk + 1) * IBLK)
                            nc.tensor.matmul(
                                op[:, sl], vt[:, j - 1, :], prev[:, sl],
                                start=(j == 1), stop=False,
                            )
                    tail_step(h, j)
                    if j == 0:
                        nc.vector.tensor_copy(acch[h][:], pt[:])
                    else:
                        nc.vector.tensor_add(acch[h][:], acch[h][:], pt[:])
                    prev = pt
                for kk in range(4):
                    sl = slice(kk * IBLK, (kk + 1) * IBLK)
                    nc.tensor.matmul(
                        op[:, sl], vt[:, NJ - 1, :], prev[:, sl],
                        start=False, stop=True,
                    )
                if h == 0:
                    nc.vector.tensor_copy(o_sbh[h][:], op[:])
                else:
                    # h1's copy sits on the exposed tail: ACT is idle after
                    # its last exp while DVE still has the final add, the
                    # recips and the o_norm muls queued
                    nc.scalar.activation(o_sbh[h][:], op[:], AF.Copy)

            # h1 tail (h0's was interleaved above)
            emit_denom(1, 0)
            emit_denom(1, 1)
            for ib in range(4, 8):
                emit_proj(ib, 1, (ib - 4) * IBLK)

    _dedup_ldweights(nc, __import__('concourse.mybir', fromlist=['mybir']))
    nc.compile()
    return nc


def _get_nc():
    if "nc" not in _CACHE:
        _CACHE["nc"] = _build()
    return _CACHE["nc"]


def _prep_inputs(x, gamma, beta, w_qkv, b_qkv, w_proj, b_proj):
    x = np.ascontiguousarray(x, dtype=np.float32)
    w_qkv = np.asarray(w_qkv, dtype=np.float32)
    b_qkv = np.asarray(b_qkv, dtype=np.float32)
    w_proj = np.asarray(w_proj, dtype=np.float32)
    b_proj = np.asarray(b_proj, dtype=np.float32)

    wq = w_qkv[0:C, :]
    wk = w_qkv[C:2 * C, :]
    wv = w_qkv[2 * C:3 * C, :]
    bqv = b_qkv[0:C]
    bvv = b_qkv[2 * C:3 * C]

    wqT = np.ascontiguousarray((wq * SCALE).T)
    wkT = np.ascontiguousarray(wk.T)
    wvT = np.ascontiguousarray(wv.T)
    wpT = np.ascontiguousarray(w_proj.T)
    beff = (b_proj + w_proj @ bvv).astype(np.float32)

    ig = np.zeros((C, GROUPS), np.float32)
    ig[np.arange(C), np.arange(C) // GSIZE] = 1.0
    igt = np.ascontiguousarray(ig.T)

    common = {
        "gamma": np.asarray(gamma, np.float32).reshape(C, 1),
        "beta": np.asarray(beta, np.float32).reshape(C, 1),
        "bq": (bqv * SCALE).reshape(C, 1),
        "beff": beff.reshape(C, 1),
        "wqT": wqT,
        "wkT": wkT,
        "wvT": wvT,
        "wpT": wpT,
        "ig": ig,
        "igt": igt,
    }
    in_maps = []
    for b in range(B):
        m = dict(common)
        m["x"] = np.ascontiguousarray(x[b].reshape(C, HW))
        in_maps.append(m)
    return in_maps


def kernel(x, gamma, beta, w_qkv, b_qkv, w_proj, b_proj):
    from concourse.bass_utils import run_bass_kernel_spmd

    nc = _get_nc()
    in_maps = _prep_inputs(x, gamma, beta, w_qkv, b_qkv, w_proj, b_proj)
    res = run_bass_kernel_spmd(nc, in_maps, list(range(B)))
    out = np.stack([res.results[b]["out"] for b in range(B)], axis=0)
    return out.reshape(B, C, H, W).astype(np.float32)
